# revision 20
# baseline (speedup 1.0000x reference)
"""Stress-majorization loss kernel for Trainium2 (8 NeuronCores).

Problem: pos [8192,2] f32, dist [8192,8192] f32 ->
    scalar sum of ((|p_i - p_j| - d_ij)/d_ij)^2 over entries with d_ij != 0.

Strategy (per-core row sharding, 1024 rows each):
 - Host: replace d==0 entries by 2^50 (each then contributes exactly 1.0,
   subtracted via the host-side zero count), and factor the squared pairwise
   distances so PE computes sq_ij = |p_i - p_j|^2 + EPS as a matmul:
     a_i = [1, n_i+EPS, -2x_i, -2y_i],  b_j = [n_j, 1, x_j, y_j]
   Each fp32 component is split into 3 bf16 terms; the 6 dominant term-pair
   products form a K=24 bf16 matmul (error ~1e-7, full bf16 PE rate).
 - Device, per [128,8192] row-tile:
     DVE: rd = reciprocal_approx_fast(d)      (in place over d)
     PE:  sq -> PSUM (4 chunks x 4 matmuls of 512 cols, K=24 bf16)
     ACT: pred = sqrt(psum)                   (table set: sqrt_and_others)
     DVE/GPSIMD: w = pred * rd                (column-split 1/3 : 2/3)
     ACT: square(w, bias=-1, accum_out) -> per-partition partial sums
   Final: reduce partials, cross-partition sum via ones-matmul, DMA out.
 - Host: total = sum(core partials) - (#zeros in dist).
"""
import sys
sys.path.insert(0, "/opt/trn_rl_repo")

import numpy as np
import ml_dtypes

N = 8192
NCORES = 8
ROWS_PER_CORE = N // NCORES          # 1024
RTILES = ROWS_PER_CORE // 128        # 8 row tiles of 128
CHUNK = 2048                         # PSUM chunk (4 banks)
MMF = 512                            # matmul free dim (1 PSUM bank)
KB = 4                               # base contraction dim
NPAIR = 6                            # bf16 split term-pairs kept
K = KB * NPAIR                       # 24
DVE_CCOLS = 800                      # per-chunk w-columns on DVE; rest GPSIMD
EPS = np.float32(4e-6)               # keeps PSUM sq > 0 despite cancellation
BIG = np.float32(2.0 ** 50)          # sentinel for d==0 entries

_cache = {}


def _build_nc():
    import concourse.bacc as bacc
    import concourse.mybir as mybir
    import concourse.tile as tile

    f32 = mybir.dt.float32
    bf16 = mybir.dt.bfloat16
    A = mybir.ActivationFunctionType
    OP = mybir.AluOpType

    nc = bacc.Bacc("TRN2", target_bir_lowering=False, debug=False)
    dists = nc.dram_tensor("dists", [ROWS_PER_CORE, N], f32, kind="ExternalInput")
    acore = nc.dram_tensor("acore", [K, ROWS_PER_CORE], bf16, kind="ExternalInput")
    bfull = nc.dram_tensor("bfull", [K, N], bf16, kind="ExternalInput")
    out = nc.dram_tensor("out", [1, 1], f32, kind="ExternalOutput")

    with tile.TileContext(nc) as tc:
        with tc.tile_pool(name="small", bufs=1) as small, \
             tc.tile_pool(name="d0pool", bufs=4) as d0pool, \
             tc.tile_pool(name="dpool", bufs=2) as dpool, \
             tc.tile_pool(name="prpool", bufs=2) as prpool, \
             tc.tile_pool(name="psum", bufs=2, space="PSUM") as psp:

            NCH = N // CHUNK
            t_acc = small.tile([128, RTILES * NCH], f32)
            t_neg1 = small.tile([128, 1], f32)
            t_ones = small.tile([128, 1], f32)

            # row 0's d chunks first: the reciprocal stream is the critical
            # engine, so its first input must land as early as possible
            t_d0s = []
            for q in range(NCH):
                c0 = q * CHUNK
                t_d0q = d0pool.tile([128, CHUNK], f32, tag="d0")
                nc.sync.dma_start(t_d0q[:], dists[0:128, c0:c0 + CHUNK])
                nc.vector.reciprocal_approx_fast(t_d0q[:], t_d0q[:])
                t_d0s.append(t_d0q)

            t_a = small.tile([K, ROWS_PER_CORE], bf16)
            t_b = small.tile([K, N], bf16)
            nc.sync.dma_start(t_a[:], acore[:])
            nc.sync.dma_start(t_b[:], bfull[:])
            nc.vector.memset(t_neg1[:], -1.0)
            nc.vector.memset(t_ones[:], 1.0)

            for r in range(RTILES):
                lhsT = t_a[:, r * 128:(r + 1) * 128]
                if r == 0:
                    t_ds = [t[:] for t in t_d0s]
                else:
                    t_d = dpool.tile([128, N], f32, tag="d")
                    nc.sync.dma_start(
                        t_d[:], dists[r * 128:(r + 1) * 128, :])
                    # in-place masked reciprocal (no zeros/denorms in input)
                    nc.vector.reciprocal_approx_fast(t_d[:], t_d[:])
                    t_ds = [t_d[:, q * CHUNK:(q + 1) * CHUNK]
                            for q in range(NCH)]

                t_pred = prpool.tile([128, N], f32, tag="pred")
                for q in range(NCH):
                    c0 = q * CHUNK
                    t_ps = psp.tile([128, CHUNK], f32, tag="ps")
                    for j in range(CHUNK // MMF):
                        col = c0 + j * MMF
                        nc.tensor.matmul(
                            t_ps[:, j * MMF:(j + 1) * MMF],
                            lhsT,
                            t_b[:, col:col + MMF],
                            start=True, stop=True)
                    nc.scalar.activation(
                        t_pred[:, c0:c0 + CHUNK], t_ps[:], A.Sqrt)

                # w = pred * rd, in place over pred (chunked so each square
                # waits only on its own chunk's multiply)
                for q in range(NCH):
                    c0, c1 = q * CHUNK, (q + 1) * CHUNK
                    nc.vector.tensor_tensor(
                        t_pred[:, c0:c1], t_pred[:, c0:c1],
                        t_ds[q], OP.mult)
                for q in range(NCH):
                    c0, c1 = q * CHUNK, (q + 1) * CHUNK
                    nc.scalar.activation(
                        t_pred[:, c0:c1], t_pred[:, c0:c1], A.Square,
                        bias=t_neg1[:], scale=1.0,
                        accum_out=t_acc[:, r * NCH + q:r * NCH + q + 1])

            # reduce the per-row-tile partials, then sum across partitions
            t_red = small.tile([128, 1], f32)
            nc.vector.tensor_reduce(t_red[:], t_acc[:], mybir.AxisListType.X, OP.add)
            t_fin_full = psp.tile([128, CHUNK], f32, tag="ps")
            t_fin = t_fin_full[:1, :1]
            nc.tensor.matmul(t_fin, t_ones[:], t_red[:], start=True, stop=True)
            t_out = small.tile([1, 1], f32)
            nc.vector.tensor_copy(t_out[:], t_fin)
            nc.sync.dma_start(out[:], t_out[:])

    nc.compile()
    return nc


def _split3(v: np.ndarray):
    """Split fp32 vector into 3 bf16 terms summing to v (error ~2^-27 |v|)."""
    v = v.astype(np.float32)
    v0 = v.astype(ml_dtypes.bfloat16)
    r1 = v - v0.astype(np.float32)
    v1 = r1.astype(ml_dtypes.bfloat16)
    r2 = r1 - v1.astype(np.float32)
    v2 = r2.astype(ml_dtypes.bfloat16)
    return v0, v1, v2


def _prep_inputs(pos: np.ndarray, dist: np.ndarray):
    assert pos.shape == (N, 2) and dist.shape == (N, N)
    pos = np.ascontiguousarray(pos, dtype=np.float32)
    dist = np.ascontiguousarray(dist, dtype=np.float32)

    # host-side mask prep: d==0 -> BIG sentinel (device yields exactly 1.0 per
    # such entry: w = pred/BIG ~ 1e-15, (w-1)^2 rounds to 1.0 in fp32)
    zmask = dist == 0.0
    nzeros = int(np.count_nonzero(zmask))
    dist_safe = np.where(zmask, BIG, dist)

    x = pos[:, 0].astype(np.float64)
    y = pos[:, 1].astype(np.float64)
    n = x * x + y * y
    a_full32 = np.stack([np.ones(N), n + np.float64(EPS), -2.0 * x, -2.0 * y]
                        ).astype(np.float32)          # [4, N]
    b_full32 = np.stack([n, np.ones(N), x, y]).astype(np.float32)  # [4, N]

    a0, a1, a2 = _split3(a_full32)
    b0, b1, b2 = _split3(b_full32)
    # term pairs kept: (a0,b0) (a0,b1) (a1,b0) (a0,b2) (a2,b0) (a1,b1)
    a_parts = [a0, a0, a1, a0, a2, a1]
    b_parts = [b0, b1, b0, b2, b0, b1]
    a_full = np.concatenate(a_parts, axis=0)   # [24, N] bf16
    b_full = np.concatenate(b_parts, axis=0)   # [24, N] bf16

    in_maps = []
    for c in range(NCORES):
        r0 = c * ROWS_PER_CORE
        in_maps.append({
            "dists": dist_safe[r0:r0 + ROWS_PER_CORE, :],
            "acore": np.ascontiguousarray(a_full[:, r0:r0 + ROWS_PER_CORE]),
            "bfull": b_full,
        })
    return in_maps, nzeros


def kernel(pos: np.ndarray, dist: np.ndarray) -> np.ndarray:
    from concourse.bass_utils import run_bass_kernel_spmd

    in_maps, nzeros = _prep_inputs(pos, dist)
    if "nc" not in _cache:
        _cache["nc"] = _build_nc()
    nc = _cache["nc"]

    res = run_bass_kernel_spmd(nc, in_maps, list(range(NCORES)))
    partials = [float(res.results[c]["out"][0, 0]) for c in range(NCORES)]
    total = sum(partials) - float(nzeros)
    return np.array(total, dtype=np.float32)


# revision 22
# speedup vs baseline: 1.2215x; 1.2215x over previous
"""Stress-majorization loss kernel for Trainium2 (8 NeuronCores).

Problem: pos [8192,2] f32, dist [8192,8192] f32 ->
    scalar sum of ((|p_i - p_j| - d_ij)/d_ij)^2 over entries with d_ij != 0.

Strategy (per-core row sharding, 1024 rows each):
 - Host: replace d==0 entries by 2^50 (each then contributes exactly 1.0,
   subtracted via the host-side zero count), and factor the squared pairwise
   distances so PE computes sq_ij = |p_i - p_j|^2 + EPS as a matmul:
     a_i = [1, n_i+EPS, -2x_i, -2y_i],  b_j = [n_j, 1, x_j, y_j]
   Each fp32 component is split into 3 bf16 terms; the 6 dominant term-pair
   products form a K=24 bf16 matmul (error ~1e-7, full bf16 PE rate).
 - Device, per [128,8192] row-tile:
     DVE: rd = reciprocal_approx_fast(d)      (in place over d)
     PE:  sq -> PSUM (4 chunks x 4 matmuls of 512 cols, K=24 bf16)
     ACT: pred = sqrt(psum)                   (table set: sqrt_and_others)
     DVE/GPSIMD: w = pred * rd                (column-split 1/3 : 2/3)
     ACT: square(w, bias=-1, accum_out) -> per-partition partial sums
   Final: reduce partials, cross-partition sum via ones-matmul, DMA out.
 - Host: total = sum(core partials) - (#zeros in dist).
"""
import sys
sys.path.insert(0, "/opt/trn_rl_repo")

import numpy as np
import ml_dtypes

N = 8192
NCORES = 8
ROWS_PER_CORE = N // NCORES          # 1024
RTILES = ROWS_PER_CORE // 128        # 8 row tiles of 128
CHUNK = 2048                         # PSUM chunk (4 banks)
MMF = 512                            # matmul free dim (1 PSUM bank)
KB = 4                               # base contraction dim
NPAIR = 6                            # bf16 split term-pairs kept
K = KB * NPAIR                       # 24
DVE_CCOLS = 800                      # per-chunk w-columns on DVE; rest GPSIMD
EPS = np.float32(4e-6)               # keeps PSUM sq > 0 despite cancellation
BIG = np.float32(2.0 ** 50)          # sentinel for d==0 entries

_cache = {}


def _build_nc():
    import concourse.bacc as bacc
    import concourse.mybir as mybir
    import concourse.tile as tile

    f32 = mybir.dt.float32
    bf16 = mybir.dt.bfloat16
    A = mybir.ActivationFunctionType
    OP = mybir.AluOpType

    nc = bacc.Bacc("TRN2", target_bir_lowering=False, debug=False)
    dists = nc.dram_tensor("dists", [ROWS_PER_CORE, N], f32, kind="ExternalInput")
    acore = nc.dram_tensor("acore", [K, ROWS_PER_CORE], bf16, kind="ExternalInput")
    bfull = nc.dram_tensor("bfull", [K, N], bf16, kind="ExternalInput")
    out = nc.dram_tensor("out", [1, 1], f32, kind="ExternalOutput")

    with tile.TileContext(nc) as tc:
        with tc.tile_pool(name="small", bufs=1) as small, \
             tc.tile_pool(name="dpool", bufs=10) as dpool, \
             tc.tile_pool(name="prpool", bufs=2) as prpool, \
             tc.tile_pool(name="psum", bufs=2, space="PSUM") as psp:

            NCH = N // CHUNK
            t_a = small.tile([K, ROWS_PER_CORE], bf16)
            t_b = small.tile([K, N], bf16)
            t_acc = small.tile([128, RTILES * NCH], f32)
            t_neg1 = small.tile([128, 1], f32)
            t_ones = small.tile([128, 1], f32)
            nc.sync.dma_start(t_a[:], acore[:])
            nc.sync.dma_start(t_b[:], bfull[:])
            nc.vector.memset(t_neg1[:], -1.0)
            nc.vector.memset(t_ones[:], 1.0)

            for r in range(RTILES):
                lhsT = t_a[:, r * 128:(r + 1) * 128]
                # per-chunk d tiles: DMA 1MB each so the reciprocal starts
                # as soon as the first chunk lands
                t_ds = []
                for q in range(NCH):
                    c0 = q * CHUNK
                    t_dq = dpool.tile([128, CHUNK], f32, tag="d")
                    nc.sync.dma_start(
                        t_dq[:], dists[r * 128:(r + 1) * 128, c0:c0 + CHUNK])
                    # in-place masked reciprocal (no zeros/denorms in input)
                    nc.vector.reciprocal_approx_fast(t_dq[:], t_dq[:])
                    t_ds.append(t_dq[:])

                t_pred = prpool.tile([128, N], f32, tag="pred")
                for q in range(NCH):
                    c0 = q * CHUNK
                    t_ps = psp.tile([128, CHUNK], f32, tag="ps")
                    for j in range(CHUNK // MMF):
                        col = c0 + j * MMF
                        nc.tensor.matmul(
                            t_ps[:, j * MMF:(j + 1) * MMF],
                            lhsT,
                            t_b[:, col:col + MMF],
                            start=True, stop=True)
                    nc.scalar.activation(
                        t_pred[:, c0:c0 + CHUNK], t_ps[:], A.Sqrt)

                # w = pred * rd, in place over pred (chunked so each square
                # waits only on its own chunk's multiply)
                for q in range(NCH):
                    c0, c1 = q * CHUNK, (q + 1) * CHUNK
                    nc.vector.tensor_tensor(
                        t_pred[:, c0:c1], t_pred[:, c0:c1],
                        t_ds[q], OP.mult)
                for q in range(NCH):
                    c0, c1 = q * CHUNK, (q + 1) * CHUNK
                    nc.scalar.activation(
                        t_pred[:, c0:c1], t_pred[:, c0:c1], A.Square,
                        bias=t_neg1[:], scale=1.0,
                        accum_out=t_acc[:, r * NCH + q:r * NCH + q + 1])

            # reduce the per-row-tile partials, then sum across partitions
            t_red = small.tile([128, 1], f32)
            nc.vector.tensor_reduce(t_red[:], t_acc[:], mybir.AxisListType.X, OP.add)
            t_fin_full = psp.tile([128, CHUNK], f32, tag="ps")
            t_fin = t_fin_full[:1, :1]
            nc.tensor.matmul(t_fin, t_ones[:], t_red[:], start=True, stop=True)
            t_out = small.tile([1, 1], f32)
            nc.vector.tensor_copy(t_out[:], t_fin)
            nc.sync.dma_start(out[:], t_out[:])

    nc.compile()
    return nc


def _split3(v: np.ndarray):
    """Split fp32 vector into 3 bf16 terms summing to v (error ~2^-27 |v|)."""
    v = v.astype(np.float32)
    v0 = v.astype(ml_dtypes.bfloat16)
    r1 = v - v0.astype(np.float32)
    v1 = r1.astype(ml_dtypes.bfloat16)
    r2 = r1 - v1.astype(np.float32)
    v2 = r2.astype(ml_dtypes.bfloat16)
    return v0, v1, v2


def _prep_inputs(pos: np.ndarray, dist: np.ndarray):
    assert pos.shape == (N, 2) and dist.shape == (N, N)
    pos = np.ascontiguousarray(pos, dtype=np.float32)
    dist = np.ascontiguousarray(dist, dtype=np.float32)

    # host-side mask prep: d==0 -> BIG sentinel (device yields exactly 1.0 per
    # such entry: w = pred/BIG ~ 1e-15, (w-1)^2 rounds to 1.0 in fp32)
    zmask = dist == 0.0
    nzeros = int(np.count_nonzero(zmask))
    dist_safe = np.where(zmask, BIG, dist)

    x = pos[:, 0].astype(np.float64)
    y = pos[:, 1].astype(np.float64)
    n = x * x + y * y
    a_full32 = np.stack([np.ones(N), n + np.float64(EPS), -2.0 * x, -2.0 * y]
                        ).astype(np.float32)          # [4, N]
    b_full32 = np.stack([n, np.ones(N), x, y]).astype(np.float32)  # [4, N]

    a0, a1, a2 = _split3(a_full32)
    b0, b1, b2 = _split3(b_full32)
    # term pairs kept: (a0,b0) (a0,b1) (a1,b0) (a0,b2) (a2,b0) (a1,b1)
    a_parts = [a0, a0, a1, a0, a2, a1]
    b_parts = [b0, b1, b0, b2, b0, b1]
    a_full = np.concatenate(a_parts, axis=0)   # [24, N] bf16
    b_full = np.concatenate(b_parts, axis=0)   # [24, N] bf16

    in_maps = []
    for c in range(NCORES):
        r0 = c * ROWS_PER_CORE
        in_maps.append({
            "dists": dist_safe[r0:r0 + ROWS_PER_CORE, :],
            "acore": np.ascontiguousarray(a_full[:, r0:r0 + ROWS_PER_CORE]),
            "bfull": b_full,
        })
    return in_maps, nzeros


def kernel(pos: np.ndarray, dist: np.ndarray) -> np.ndarray:
    from concourse.bass_utils import run_bass_kernel_spmd

    in_maps, nzeros = _prep_inputs(pos, dist)
    if "nc" not in _cache:
        _cache["nc"] = _build_nc()
    nc = _cache["nc"]

    res = run_bass_kernel_spmd(nc, in_maps, list(range(NCORES)))
    partials = [float(res.results[c]["out"][0, 0]) for c in range(NCORES)]
    total = sum(partials) - float(nzeros)
    return np.array(total, dtype=np.float32)
